# revision 43
# baseline (speedup 1.0000x reference)
"""Trainium2 Bass kernel for nn_ExemplarNoAttention (retrieval_knn).

logits[b,c] = log(eps + sum_{e: label[e]==c} exp(-beta * ||x_b - E_e||^2))

Sharding: data-parallel over the batch. Each of the 8 NeuronCores computes
its own 128 queries against the full exemplar bank (replicated, class-sorted
on the host); the host concatenates the per-core (128, 10) outputs. No
collectives: each core's pipeline is fully independent.

Device pipeline per core (one batch tile of 128 queries):
  TensorE : psum[b,e] = 2*beta*<x_b,E_e> - beta*e2_e  (bf16 GEMM, K=65:
            rows 0..63 = features, row 64 = 1 -> -beta*e2 augmentation row;
            one stationary weight load, 98 chunked matmuls at stream rate)
  ScalarE : exp(psum + bias_b) with bias_b = -beta*||x_b||^2. For segments
            of "ACT-route" classes the Exp carries a fused accum_out that
            yields the class-segment sum directly (f32). Other classes get
            bulk Exp into bf16 SBUF.
  VectorE : "DVE-route" class pieces: tensor_scalar with fused accumulate.
  Piece sums -> per-class sums (tiny reduce), logits = Ln(sums + eps),
  DMA out (128, 10) per core.
"""

import os
import numpy as np
import ml_dtypes

NUM_CLASSES = 10
EPS = 1e-12
N_CORES = 8
B = 1024
D = 64
NE = 50000
BT = 128
SEG_ALIGN = 32
CHUNK = 512
WIN = 2048             # psum window (4 banks)
N_ACT_CLASSES = 2      # classes whose sums come from fused Exp+accum (ScalarE)
FOLD_MIN = 700        # DVE pieces at least this long get one 2x fold first
N_SCHRAUD = 2          # windows whose exp runs on VectorE (Schraudolph bits)
SCH_A = 128.0 / float(np.log(2.0))   # bf16 exponent scale
SCH_B = 127.0 * 128.0 - 6.0          # bf16 bias + rounding tweak

LAST_EXEC_NS = None
LAST_RESULTS = None
TRACE = bool(int(os.environ.get("KERNEL_TRACE", "0")))
TRACE_DIR = os.environ.get("KERNEL_TRACE_DIR", "")


def _host_prep(x, exemplars, exemplar_labels, beta_raw):
    x = np.asarray(x, dtype=np.float32)
    E = np.asarray(exemplars, dtype=np.float32)
    labels = np.asarray(exemplar_labels).astype(np.int64)
    beta = float(np.logaddexp(0.0, np.float64(beta_raw.reshape(-1)[0])))

    # global class-sorted layout with 32-aligned per-class segments
    seg_idx = []
    seg_sizes = []
    for c in range(NUM_CLASSES):
        idx_c = np.nonzero(labels == c)[0]
        seg_idx.append(idx_c)
        seg_sizes.append(max(SEG_ALIGN, int(-(-len(idx_c) // SEG_ALIGN) * SEG_ALIGN)))
    seg_offs = np.concatenate([[0], np.cumsum(seg_sizes)]).astype(np.int64)
    e_pad = int(seg_offs[-1])

    e2 = (E.astype(np.float64) ** 2).sum(axis=1)
    ea = np.zeros((D + 1, e_pad), dtype=np.float32)
    ea[D, :] = -1.0e38  # padding slots contribute exp() == 0
    for c in range(NUM_CLASSES):
        idx = seg_idx[c]
        o = int(seg_offs[c])
        ea[:D, o:o + len(idx)] = (2.0 * beta) * E[idx].T
        ea[D, o:o + len(idx)] = (-beta * e2[idx]).astype(np.float32)
    ea = ea.astype(ml_dtypes.bfloat16)

    # per-core stationary x tiles and activation biases
    xa = np.ones((D + 1, B), dtype=np.float32)
    xa[:D, :] = x.T
    xa = xa.astype(ml_dtypes.bfloat16)
    x2 = (x.astype(np.float64) ** 2).sum(axis=1)
    bias = (-beta * x2).astype(np.float32)

    xa_cores = [np.ascontiguousarray(xa[:, i * BT:(i + 1) * BT]) for i in range(N_CORES)]
    bias_cores = [
        np.ascontiguousarray(bias[i * BT:(i + 1) * BT].reshape(BT, 1))
        for i in range(N_CORES)
    ]
    # per-partition Schraudolph affine: n = SCH_A*psum + (SCH_A*bias + SCH_B)
    schb_cores = [
        np.ascontiguousarray(
            (SCH_A * bias[i * BT:(i + 1) * BT].astype(np.float64) + SCH_B)
            .astype(np.float32)
            .reshape(BT, 1)
        )
        for i in range(N_CORES)
    ]
    return ea, xa_cores, bias_cores, schb_cores, seg_offs, seg_sizes, e_pad


def _build_program(seg_offs, seg_sizes, e_pad):
    from contextlib import ExitStack
    import concourse.bass as bass
    import concourse.tile as tile
    from concourse import bacc, mybir
    import bass_rust

    f32 = mybir.dt.float32
    bf16 = mybir.dt.bfloat16

    class _Bacc(bacc.Bacc):
        # Force Exp and Ln onto the one table set that holds both, so the
        # kernel pays a single ACT_TABLE_LOAD instead of an exp-set load at
        # the start plus an ln-set load on the critical tail. Table ids are
        # positional, so positions are kept and only the choosable functions
        # are masked.
        def insert_act_table_loads(self):
            from concourse.hw_specs import get_activation_tables

            has_activation = any(
                isinstance(i, mybir.InstActivation)
                for b in self.main_func.blocks
                for i in b.instructions
            )
            if not has_activation:
                return
            E = mybir.ActivationFunctionType.Exp
            L = mybir.ActivationFunctionType.Ln
            tables = []
            for name, fns in get_activation_tables(self.m.arch).items():
                if name != "natural_log_exp_and_others":
                    fns = fns - {E, L}
                tables.append((name, fns))
            bass_rust.insert_act_table_loads(self, tables)

    nc = _Bacc(
        "TRN2",
        target_bir_lowering=False,
        debug=False,
        enable_asserts=False,
        num_devices=N_CORES,
    )

    ea_d = nc.dram_tensor("ea", [D + 1, e_pad], bf16, kind="ExternalInput").ap()
    xa_d = nc.dram_tensor("xa", [D + 1, BT], bf16, kind="ExternalInput").ap()
    bias_d = nc.dram_tensor("biasx", [BT, 1], f32, kind="ExternalInput").ap()
    schb_d = nc.dram_tensor("schb", [BT, 1], f32, kind="ExternalInput").ap()
    out_d = nc.dram_tensor("logits", [BT, NUM_CLASSES], f32, kind="ExternalOutput").ap()

    # windows of <= WIN columns; pieces = (class, window) intersections
    wins = []
    o = 0
    while o < e_pad:
        wins.append((o, min(WIN, e_pad - o)))
        o += WIN
    dve_start = int(seg_offs[N_ACT_CLASSES])
    dve_len = e_pad - dve_start

    # windows containing only DVE-route classes are Schraudolph candidates
    cand = []
    for wi, (wo, wl) in enumerate(wins):
        if wo >= dve_start and 2 <= wi < len(wins) - 2:
            cand.append(wi)
    sch_wins = set()
    if N_SCHRAUD and cand:
        idxs = np.linspace(0.3 * (len(cand) - 1), 0.75 * (len(cand) - 1), num=N_SCHRAUD)
        sch_wins = {cand[int(round(i))] for i in idxs}

    # piece table per window: (class, col_off, col_len)
    win_pieces = []
    for (wo, wl) in wins:
        pieces = []
        for c in range(NUM_CLASSES):
            lo = max(int(seg_offs[c]), wo)
            hi = min(int(seg_offs[c + 1]), wo + wl)
            if lo < hi:
                pieces.append((c, lo, hi - lo))
        win_pieces.append(pieces)
    n_pieces_per_class = [0] * NUM_CLASSES
    piece_col = {}  # (c, lo) -> column in piece-sum tile
    pcol = 0
    for pieces in win_pieces:
        for (c, lo, ln) in pieces:
            piece_col[(c, lo)] = pcol
            n_pieces_per_class[c] += 1
            pcol += 1
    n_pieces = pcol
    class_piece_range = []
    acc = 0
    for c in range(NUM_CLASSES):
        class_piece_range.append((acc, acc + n_pieces_per_class[c]))
        acc += n_pieces_per_class[c]
    class_last_win = [0] * NUM_CLASSES
    for wi, pieces in enumerate(win_pieces):
        for (c, lo, ln) in pieces:
            class_last_win[c] = wi

    with tile.TileContext(nc) as tc, ExitStack() as ctx:
        const_pool = ctx.enter_context(tc.tile_pool(name="const", bufs=1))
        psum_pool = ctx.enter_context(tc.tile_pool(name="psum", bufs=2, space="PSUM"))
        sims_pool = ctx.enter_context(tc.tile_pool(name="sims", bufs=1))
        work_pool = ctx.enter_context(tc.tile_pool(name="work", bufs=1))
        ja_pool = ctx.enter_context(tc.tile_pool(name="ja", bufs=2))
        jd_pool = ctx.enter_context(tc.tile_pool(name="jd", bufs=2))
        fold_pool = ctx.enter_context(tc.tile_pool(name="fold", bufs=2))

        xa_t = const_pool.tile([D + 1, BT], bf16, name="xa_t")
        nc.sync.dma_start(out=xa_t[:], in_=xa_d[:])
        bias_t = const_pool.tile([BT, 1], f32, name="bias_t")
        nc.sync.dma_start(out=bias_t[:], in_=bias_d[:])
        schb_t = const_pool.tile([BT, 1], f32, name="schb_t")
        nc.sync.dma_start(out=schb_t[:], in_=schb_d[:])
        eps_t = const_pool.tile([BT, 1], f32, name="eps_t")
        nc.vector.memset(eps_t[:], float(EPS))

        # ea lives in a few group tiles (fewer tiles -> fewer release sems in
        # the kernel tail); each group is DMA'd in window-sized chunks
        EA_GROUP = 5
        ea_w = [None] * len(wins)
        gi = 0
        while gi < len(wins):
            g = wins[gi:gi + EA_GROUP]
            g_off = g[0][0]
            g_len = sum(wl for (_, wl) in g)
            t_ = const_pool.tile(
                [D + 1, g_len], bf16, name=f"ea_g{g_off}", tag=f"ea_g{g_off}"
            )
            for (wo, wl) in g:
                step = CHUNK if (gi == 0 and wo == 0) else wl
                co = 0
                while co < wl:
                    cl = min(step, wl - co)
                    nc.sync.dma_start(
                        out=t_[:, wo - g_off + co:wo - g_off + co + cl],
                        in_=ea_d[:, wo + co:wo + co + cl],
                    )
                    co += cl
            for k, (wo, wl) in enumerate(g):
                ea_w[gi + k] = t_[:, wo - g_off:wo - g_off + wl]
            gi += EA_GROUP

        sims = sims_pool.tile([BT, dve_len], bf16, name="sims")
        pieces_t = work_pool.tile([BT, max(n_pieces, 1)], f32, name="pieces_t")
        cls = work_pool.tile([BT, NUM_CLASSES], f32, name="clst")
        junkf = work_pool.tile([BT, max(n_pieces, 1)], f32, name="junkf")

        for wi, (wo, wl) in enumerate(wins):
            ps = psum_pool.tile([BT, WIN], f32, tag="ps")
            co = 0
            while co < wl:
                cl = min(CHUNK, wl - co)
                nc.tensor.matmul(
                    ps[:, co:co + cl],
                    lhsT=xa_t[:],
                    rhs=ea_w[wi][:, co:co + cl],
                    start=True,
                    stop=True,
                )
                co += cl
            # ACT-route pieces: fused exp + accumulate straight to piece sum
            for (c, lo, ln) in win_pieces[wi]:
                if c < N_ACT_CLASSES:
                    pc = piece_col[(c, lo)]
                    ja = ja_pool.tile([BT, WIN], bf16, tag="ja")
                    nc.scalar.activation(
                        ja[:, :ln],
                        ps[:, lo - wo:lo - wo + ln],
                        mybir.ActivationFunctionType.Exp,
                        bias=bias_t[:, 0:1],
                        scale=1.0,
                        accum_out=pieces_t[:, pc:pc + 1],
                    )
            # DVE-route region of this window: one bulk exp into bf16 sims
            dlo = max(wo, dve_start)
            if dlo < wo + wl:
                ln = wo + wl - dlo
                dst = sims[:, dlo - dve_start:dlo - dve_start + ln]
                if wi in sch_wins:
                    # Schraudolph: bf16 bits = uint16(SCH_A*z + SCH_B);
                    # saturation at 0 doubles as the exp() underflow clamp
                    nc.vector.tensor_scalar(
                        dst.bitcast(mybir.dt.uint16),
                        ps[:, dlo - wo:dlo - wo + ln],
                        float(SCH_A),
                        schb_t[:, 0:1],
                        mybir.AluOpType.mult,
                        mybir.AluOpType.add,
                    )
                else:
                    nc.scalar.activation(
                        dst,
                        ps[:, dlo - wo:dlo - wo + ln],
                        mybir.ActivationFunctionType.Exp,
                        bias=bias_t[:, 0:1],
                        scale=1.0,
                    )
            for (c, lo, ln) in win_pieces[wi]:
                if c >= N_ACT_CLASSES:
                    pc = piece_col[(c, lo)]
                    so = lo - dve_start
                    jd = jd_pool.tile([BT, WIN], bf16, tag="jd")
                    src = sims
                    if ln >= FOLD_MIN and ln % 2 == 0:
                        h = ln // 2
                        fold = fold_pool.tile([BT, WIN // 2], bf16, tag="fold")
                        nc.vector.tensor_add(
                            fold[:, :h], src[:, so:so + h], src[:, so + h:so + ln]
                        )
                        src, so, ln = fold, 0, h
                    nc.vector.tensor_scalar(
                        jd[:, :ln],
                        src[:, so:so + ln],
                        1.0,
                        None,
                        mybir.AluOpType.mult,
                        mybir.AluOpType.add,
                        accum_out=pieces_t[:, pc:pc + 1],
                    )
        # combine piece sums into class sums
        for c in range(NUM_CLASSES):
            plo, phi = class_piece_range[c]
            if phi - plo > 1:
                nc.vector.tensor_scalar(
                    junkf[:, plo:phi],
                    pieces_t[:, plo:phi],
                    1.0,
                    None,
                    mybir.AluOpType.mult,
                    mybir.AluOpType.add,
                    accum_out=cls[:, c:c + 1],
                )
            else:
                nc.vector.tensor_copy(cls[:, c:c + 1], pieces_t[:, plo:plo + 1])

        logit = work_pool.tile([BT, NUM_CLASSES], f32, name="logit")
        nc.scalar.activation(
            logit[:],
            cls[:],
            mybir.ActivationFunctionType.Ln,
            bias=eps_t[:, 0:1],
            scale=1.0,
        )
        nc.sync.dma_start(out=out_d, in_=logit[:])

    nc.compile()
    return nc


def kernel(x, exemplars, exemplar_labels, beta_raw):
    global LAST_EXEC_NS, LAST_RESULTS
    from concourse.bass_utils import run_bass_kernel_spmd

    ea, xa_cores, bias_cores, schb_cores, seg_offs, seg_sizes, e_pad = _host_prep(
        x, exemplars, exemplar_labels, beta_raw
    )
    nc = _build_program(seg_offs, seg_sizes, e_pad)

    in_maps = [
        {"ea": ea, "xa": xa_cores[i], "biasx": bias_cores[i], "schb": schb_cores[i]}
        for i in range(N_CORES)
    ]
    kwargs = {}
    if TRACE:
        kwargs["trace"] = True
        if TRACE_DIR:
            os.makedirs(TRACE_DIR, exist_ok=True)
            kwargs["tmpdir"] = TRACE_DIR
    ret = run_bass_kernel_spmd(nc, in_maps, list(range(N_CORES)), **kwargs)
    LAST_EXEC_NS = ret.exec_time_ns
    LAST_RESULTS = ret
    out = np.concatenate(
        [np.asarray(ret.results[i]["logits"], dtype=np.float32) for i in range(N_CORES)],
        axis=0,
    )
    return np.ascontiguousarray(out)
